# revision 1
# baseline (speedup 1.0000x reference)
"""DeepseekV4 Mega-MoE experts layer on 8 Trainium2 NeuronCores.

Strategy (expert-parallel, per sharding hint):
  - 16 experts sharded 2-per-core across 8 cores; each core receives its two
    experts' weights (losslessly converted: mxfp4*ue8m0 dequant values are
    exactly representable in TRN fp8_e4m3 for both w13 and w2).
  - Staging fp8 quantization of hidden_states runs on the host (direct
    fp32->fp8e4 cast, 1/4 the DMA bytes of fp32).
  - Tokens are gathered per expert on-device with a one-hot matmul (the
    "all-to-all"); the host sums the per-expert outputs (the "combine").

Per-core device pipeline:
  x8, g one-hot -> xgT[d,slot] via PE DoubleRow, interleaved with mm1[0]
  hT[f,tok] = mm1: lhsT=w13T chunks, rhs=xgT chunks (fp8 DR, accum over d);
    gate pass streams the w13 DMA (even k then odd k: PSUM accumulation
    groups that are concurrently open must not share a 2KB bank), up pass
    re-reads SBUF k-pair-outer into per-pair PSUM tiles.
  a^T = Silu(hT_gate) * hT_up * 2^-9, split hi+lo into TWO fp8 tensors
    (deq(hi)+deq(lo) carries ~8 mantissa bits; rel err beats bf16) so mm2
    runs fp8 DoubleRow at 2x bf16 throughput. Engines: Silu on ACT, the
    scaled multiply on DVE, hi/lo casts on GPSIMD so the mm2 PSUM->SBUF
    copies (ACT/DVE, fused per-token comb*2^9 scale) are never blocked.
  ye[tok,d] = mm2: lhsT=aT_hi/aT_lo, rhs=w2T (one PSUM accumulation per
    512-wide full-bank group, ~6-deep buffer ring across retired PSUM
    tags); bf16 half-row output DMAs drain as soon as each pair of copies
    lands.
  DMA order: x, g, comb, w13[0], w13[1], w2[0], w2[1] -- every transfer is
    consumed as it lands; the tail is mm2[1] streaming behind the last w2
    chunks. mm1 gate groups stream the w13 DMA concurrently with the first
    up-pair (8 open PSUM accumulation groups, one per bank); remaining up
    pairs re-read SBUF. TimelineSim: 43.15us/core vs 78.5us baseline.
"""

import sys

if "/opt/trn_rl_repo" not in sys.path:
    sys.path.insert(0, "/opt/trn_rl_repo")

import numpy as np
import ml_dtypes

T, D, I, E, TOPK, GROUP = 512, 2048, 768, 16, 8, 32
N_CORES = 8
E_LOC = E // N_CORES  # experts per core
S_A = 2.0 ** -9       # fixed pre-scale for fp8 hi/lo split of activations

FP8 = ml_dtypes.float8_e4m3      # TRN FP8_EXP4 (max 240) == bass dt.float8e4
BF16 = ml_dtypes.bfloat16

_FP4_TABLE = np.array(
    [0.0, 0.5, 1.0, 1.5, 2.0, 3.0, 4.0, 6.0,
     -0.0, -0.5, -1.0, -1.5, -2.0, -3.0, -4.0, -6.0], dtype=np.float32)


def _dequant_mxfp4(w_packed, sf):
    lo = _FP4_TABLE[w_packed & 0xF]
    hi = _FP4_TABLE[(w_packed >> 4) & 0xF]
    w = np.stack([lo, hi], axis=-1).reshape(*w_packed.shape[:-1], -1)
    s = (sf.astype(np.uint32) << 23).view(np.float32)
    w = w.reshape(*sf.shape, GROUP) * s[..., None]
    return w.reshape(*w_packed.shape[:-1], 2 * w_packed.shape[-1])


_PROGRAM_CACHE = {}


def _build_program(cap, split_waits=True, debug=False):
    import concourse.bass as bass
    import concourse.mybir as mybir
    import concourse.tile as tile

    _TC = tile.TileContext

    def _split_excess_waits(nc):
        # This walrus build accepts only ONE sem-wait per instruction; hoist
        # extra waits onto standalone EventSemaphore (pure-wait) instructions
        # on the same engine, which execute in order ahead of the original.
        n = 0
        for f in nc.m.functions:
            for b in f.blocks:
                out = []
                for ins in b.instructions:
                    si = ins.sync_info
                    waits = list(si.on_wait) if (si and si.on_wait) else []
                    if len(waits) > 1:
                        for k, w in enumerate(waits[:-1]):
                            out.append(mybir.InstEventSemaphore(
                                name=f"{ins.name}-xw{k}", engine=ins.engine,
                                ins=[], outs=[],
                                sync_info=mybir.SyncInfo(
                                    on_wait=[w], on_update=[])))
                            n += 1
                        si.on_wait = waits[-1:]
                    out.append(ins)
                b.instructions = out
        return n

    dt = mybir.dt
    MT = cap // 128            # token tiles per expert
    DT, IT = D // 128, I // 128      # 16, 6
    KT = 2 * I // 128                # 12 f-tiles for mm1 output
    TT = T // 128                    # 4 token chunks
    SLOTS = E_LOC * cap
    W13P, W2P = 8, 3                 # DMA parts per expert weight
    JH, KH = DT // W13P, IT // W2P   # 4 d-chunks, 2 i-chunks per part
    AF = mybir.ActivationFunctionType

    nc = bass.Bass()
    xgt_d = nc.dram_tensor("xgt", [DT, 128, SLOTS], dt.float8e4, kind="ExternalInput")
    w13_d = nc.dram_tensor("w13t", [E_LOC, DT, 128, 2 * I], dt.float8e4, kind="ExternalInput")
    w2_d = nc.dram_tensor("w2t", [E_LOC, IT, 128, D], dt.float8e4, kind="ExternalInput")
    comb_d = nc.dram_tensor("combg", [E_LOC, MT, 128, 1], dt.float32, kind="ExternalInput")
    ye_d = nc.dram_tensor("ye", [E_LOC, MT, 128, D], dt.bfloat16, kind="ExternalOutput")

    with _TC(nc) as tc:
        with (
            tc.tile_pool(name="inp", bufs=1) as inp,
            tc.tile_pool(name="wts", bufs=1) as wtsp,
            tc.tile_pool(name="xg", bufs=1) as xgp,
            tc.tile_pool(name="act", bufs=2) as actp,
            tc.tile_pool(name="at", bufs=1) as atp,
            tc.tile_pool(name="yout", bufs=1) as youtp,
            tc.tile_pool(name="ps_h", bufs=1, space="PSUM") as psh,
            tc.tile_pool(name="ps_small", bufs=2, space="PSUM") as pss,
        ):
            # ---- DMAs in consumption order on the SP ring ----
            # hidden states arrive pre-gathered and pre-transposed from the
            # host (the host already computes the routing): xgT[d, slot]
            xgT = xgp.tile([128, DT, SLOTS], dt.float8e4, tag="xgT")
            nc.sync.dma_start(xgT[:], xgt_d.rearrange("j p f -> p j f"))
            # weights: w13 both experts (mm1 order), then w2 both experts
            w13t = [[None] * W13P for _ in range(E_LOC)]
            w2t = [[None] * W2P for _ in range(E_LOC)]
            for e in range(E_LOC):
                for p in range(W13P):
                    wt = wtsp.tile([128, JH, 2 * I], dt.float8e4, tag=f"w13_{e}_{p}")
                    nc.sync.dma_start(
                        wt[:], w13_d[e, p * JH:(p + 1) * JH].rearrange("j p f -> p j f"))
                    w13t[e][p] = wt
            for e in range(E_LOC):
                for p in range(W2P):
                    w2 = wtsp.tile([128, KH, D], dt.float8e4, tag=f"w2_{e}_{p}")
                    nc.sync.dma_start(
                        w2[:], w2_d[e, p * KH:(p + 1) * KH].rearrange("k p f -> p k f"))
                    w2t[e][p] = w2
            combg = []
            for e in range(E_LOC):
                cg = inp.tile([128, MT, 1], dt.float32, tag=f"cg_{e}")
                nc.scalar.dma_start(cg[:], comb_d[e].rearrange("m p f -> p m f"))
                combg.append(cg)

            def mm1_mm(e, out_slice, half, u, k):
                p, uu = (2 * u) // JH, (2 * u) % JH
                nc.tensor.matmul(
                    out_slice,
                    w13t[e][p][:, uu:uu + 2,
                               (half * IT + k) * 128:(half * IT + k + 1) * 128],
                    xgT[:, 2 * u:2 * u + 2, e * cap:(e + 1) * cap],
                    start=(u == 0), stop=(u == DT // 2 - 1),
                    perf_mode=mybir.MatmulPerfMode.DoubleRow)

            # chain scratch (f32 staging for the hi/lo split)
            sils = actp.tile([128, IT, cap], dt.float32, tag="sil", bufs=1)
            as2s = actp.tile([128, IT, cap], dt.float32, tag="as2", bufs=1)

            # yh buffers cycle through pss plus the hp PSUM tags (free once
            # the chains have consumed them) -> 5-deep ring, so mm2 groups
            # rarely stall on the PSUM->SBUF copy latency.
            _yh_tags = ["sm", "sm", "hp0", "hp1", "hp2"]

            def mm2_mms(e, yh, aThi, aTlo, m, dq, parts):
                for p in parts:
                    for at in (aThi, aTlo):
                        nc.tensor.matmul(
                            yh[:],
                            at[:, 2 * p:2 * p + 2, m * 128:(m + 1) * 128],
                            w2t[e][p][:, 0:2, dq * 512:(dq + 1) * 512],
                            start=(p == 0 and at is aThi),
                            stop=(p == W2P - 1 and at is aTlo),
                            perf_mode=mybir.MatmulPerfMode.DoubleRow)

            def mm2_group(e, aThi, aTlo, m, dq):
                tag = _yh_tags[(m * 4 + dq) % len(_yh_tags)]
                pool = pss if tag == "sm" else psh
                yh = pool.tile([128, 512], dt.float32, tag=tag)
                mm2_mms(e, yh, aThi, aTlo, m, dq, range(W2P))
                return yh

            def ye_copy(engine, ye, yh, e, m, dq):
                if engine == 'act':
                    nc.scalar.activation(
                        ye[:, m, dq * 512:(dq + 1) * 512], yh[:],
                        AF.Copy, scale=combg[e][:, m, :])
                else:
                    nc.vector.tensor_scalar(
                        ye[:, m, dq * 512:(dq + 1) * 512], yh[:],
                        combg[e][:, m, :], None, op0=mybir.AluOpType.mult)

            def expert_front(e, hps, aThi, aTlo):
                # Merged gate/up PSUM layout: pair tile hps[p] is
                # [128, 2, 512] f32 = 2 banks; bank kk holds gate k=2p+kk in
                # its first 1KB. Up k=0,1 stream WITH the gates into the two
                # pss full-bank tiles (8 concurrently-open groups, one per
                # bank -- safe); up pairs 1,2 re-read SBUF afterwards into
                # the second 1KB of hps[1]/hps[2] banks (sequential per
                # bank). Pair 0's chain starts right at the DMA stream end
                # with no PSUM WAR gap; later pairs stagger behind the
                # re-read passes.
                ups0 = [pss.tile([128, 512], dt.float32, tag="sm",
                                 name=f"ups0_{e}_{kk}") for kk in range(2)]
                for u in range(DT // 2):
                    for k in range(IT):
                        mm1_mm(e, hps[k // 2][:, k % 2, 0:cap], 0, u, k)
                    for kk in range(2):
                        mm1_mm(e, ups0[kk][:, 0:cap], 1, u, kk)
                # pair 0 chain immediately (no ups conflict on hps[0])
                nc.scalar.activation(sils[:, 0:2, :], hps[0][:, :, 0:cap],
                                     AF.Silu)
                for kk in range(2):
                    nc.vector.scalar_tensor_tensor(
                        as2s[:, kk, :], sils[:, kk, :], S_A, ups0[kk][:, 0:cap],
                        op0=mybir.AluOpType.mult, op1=mybir.AluOpType.mult)
                nc.gpsimd.tensor_copy(aThi[:, 0:2, :], as2s[:, 0:2, :])
                nc.vector.tensor_tensor(
                    aTlo[:, 0:2, :], as2s[:, 0:2, :], aThi[:, 0:2, :],
                    op=mybir.AluOpType.subtract)
                # pairs 1,2: up re-read into hps[p] second halves, chain chases
                for p in (1, 2):
                    for kk in (0, 1):
                        for u in range(DT // 2):
                            mm1_mm(e, hps[p][:, kk, cap:2 * cap], 1, u,
                                   2 * p + kk)
                    nc.scalar.activation(
                        sils[:, 2 * p:2 * p + 2, :], hps[p][:, :, 0:cap],
                        AF.Silu)
                    nc.vector.scalar_tensor_tensor(
                        as2s[:, 2 * p:2 * p + 2, :], sils[:, 2 * p:2 * p + 2, :],
                        S_A, hps[p][:, :, cap:2 * cap],
                        op0=mybir.AluOpType.mult, op1=mybir.AluOpType.mult)
                    nc.gpsimd.tensor_copy(
                        aThi[:, 2 * p:2 * p + 2, :], as2s[:, 2 * p:2 * p + 2, :])
                    nc.vector.tensor_tensor(
                        aTlo[:, 2 * p:2 * p + 2, :], as2s[:, 2 * p:2 * p + 2, :],
                        aThi[:, 2 * p:2 * p + 2, :], op=mybir.AluOpType.subtract)

            hps0 = [psh.tile([128, 2, 2 * cap], dt.float32, tag=f"hp{p}",
                             name=f"hp{p}_0") for p in range(3)]
            aThi0 = atp.tile([128, IT, cap], dt.float8e4, tag="aThi_0")
            aTlo0 = atp.tile([128, IT, cap], dt.float8e4, tag="aTlo_0")
            expert_front(0, hps0, aThi0, aTlo0)

            hps1 = [psh.tile([128, 2, 2 * cap], dt.float32, tag=f"hp{p}",
                             name=f"hp{p}_1") for p in range(3)]
            aThi1 = atp.tile([128, IT, cap], dt.float8e4, tag="aThi_1")
            aTlo1 = atp.tile([128, IT, cap], dt.float8e4, tag="aTlo_1")
            expert_front(1, hps1, aThi1, aTlo1)

            if debug:
                dbg_xgT = nc.dram_tensor("dbg_xgT", [128, DT, SLOTS],
                                         dt.float8e4, kind="ExternalOutput")
                nc.sync.dma_start(dbg_xgT[:], xgT[:])
                dbg_hi = nc.dram_tensor("dbg_hi", [128, IT, cap],
                                        dt.float8e4, kind="ExternalOutput")
                nc.sync.dma_start(dbg_hi[:], aThi0[:])
                dbg_lo = nc.dram_tensor("dbg_lo", [128, IT, cap],
                                        dt.float8e4, kind="ExternalOutput")
                nc.sync.dma_start(dbg_lo[:], aTlo0[:])

            ye0 = youtp.tile([128, MT, D], dt.bfloat16, tag="ye_0")
            ye1 = youtp.tile([128, MT, D], dt.bfloat16, tag="ye_1")
            for e, (hi, lo, ye) in enumerate(((aThi0, aTlo0, ye0),
                                             (aThi1, aTlo1, ye1))):
                for m in range(MT):
                    for dq in range(4):
                        yh = mm2_group(e, hi, lo, m, dq)
                        ye_copy('act' if dq % 2 == 0 else 'dve', ye, yh, e, m, dq)
                        if dq % 2 == 1:  # drain per half-row for earlier outs
                            nc.sync.dma_start(
                                ye_d[e, m, :, (dq - 1) * 512:(dq + 1) * 512],
                                ye[:, m, (dq - 1) * 512:(dq + 1) * 512])

    nc.finalize()
    if split_waits:
        _split_excess_waits(nc)
    return nc


def kernel(hidden_states, topk_weights, topk_ids, w13_weight, w13_weight_scale,
           w2_weight, w2_weight_scale):
    from concourse.bass_utils import run_bass_kernel_spmd

    x = np.asarray(hidden_states, dtype=np.float32)
    tw = np.asarray(topk_weights, dtype=np.float32)
    ti = np.asarray(topk_ids)

    # host routing: combine weights + per-expert token lists
    comb = np.zeros((T, E), np.float32)
    for k in range(TOPK):
        np.add.at(comb, (np.arange(T), ti[:, k]), tw[:, k])
    routed = comb > 0.0
    idx = [np.nonzero(routed[:, e])[0] for e in range(E)]
    counts = [len(ix) for ix in idx]
    cap = max(128, -(-max(counts) // 128) * 128)

    if cap not in _PROGRAM_CACHE:
        _PROGRAM_CACHE[cap] = _build_program(cap)
    nc = _PROGRAM_CACHE[cap]

    # host staging quantization + gather/transpose (the host computes the
    # routing anyway) + lossless weight conversion
    xq8 = x.astype(FP8)
    w13 = _dequant_mxfp4(np.asarray(w13_weight), np.asarray(w13_weight_scale))
    w2 = _dequant_mxfp4(np.asarray(w2_weight), np.asarray(w2_weight_scale))
    DT, IT, TT, MT = D // 128, I // 128, T // 128, cap // 128

    in_maps = []
    for core in range(N_CORES):
        m = {}
        xgt = np.zeros((DT, 128, E_LOC * cap), FP8)
        cg = np.zeros((E_LOC, cap), np.float32)
        w13t = np.zeros((E_LOC, DT, 128, 2 * I), FP8)
        w2t = np.zeros((E_LOC, IT, 128, D), FP8)
        for le in range(E_LOC):
            e = core * E_LOC + le
            ix = idx[e]
            xgt[:, :, le * cap:le * cap + len(ix)] = \
                xq8[ix].T.reshape(DT, 128, len(ix))
            cg[le, :len(ix)] = comb[ix, e] / S_A   # undo the fp8 pre-scale
            w13t[le] = w13[e].T.astype(FP8).reshape(DT, 128, 2 * I)
            w2t[le] = w2[e].T.astype(FP8).reshape(IT, 128, D)
        m["xgt"] = xgt
        m["combg"] = np.ascontiguousarray(cg.reshape(E_LOC, MT, 128, 1))
        m["w13t"] = w13t
        m["w2t"] = w2t
        in_maps.append(m)

    res = run_bass_kernel_spmd(nc, in_maps, list(range(N_CORES)))

    out = np.zeros((T, D), np.float32)
    for core in range(N_CORES):
        ye = np.asarray(res.results[core]["ye"], dtype=np.float32).reshape(
            E_LOC, cap, D)
        for le in range(E_LOC):
            e = core * E_LOC + le
            ix = idx[e]
            out[ix] += ye[le, :len(ix)]
    return out



# revision 22
# speedup vs baseline: 1.0528x; 1.0528x over previous
"""DeepseekV4 Mega-MoE experts layer on 8 Trainium2 NeuronCores.

Strategy (expert-parallel, per sharding hint):
  - 16 experts sharded 2-per-core across 8 cores; each core receives its two
    experts' weights (losslessly converted: mxfp4*ue8m0 dequant values are
    exactly representable in TRN fp8_e4m3 for both w13 and w2).
  - Staging fp8 quantization of hidden_states runs on the host (direct
    fp32->fp8e4 cast, 1/4 the DMA bytes of fp32); the host also gathers
    tokens per expert (the "all-to-all") and sums per-expert outputs (the
    "combine").

The kernel is HBM-DMA bound (12MB/core at 360 B/ns ~= 33us of DMA busy vs
~20us PE).  v2 scheduling goals:
  - slot caps sized to the actual max token counts (rounded /16), not 256,
    cutting xgT and ye bytes;
  - one gapless DMA stream on the SP queue: comb+xgT, w13 slot0, w13 slot1,
    w2 slot0, w2 slot1, then every ye output chunk.  Outputs are emitted
    last so input transfers never wait on compute;
  - mm2 runs part-major with all 8 PSUM banks open (yh tiles alias mm1's
    hp/up banks), so after the last w2 part lands only that part's matmul
    pass (~1.7us) + copies remain before the final ye chunks stream out.

Per-core device pipeline:
  mm1 hT[f,tok] = w13T chunks x xgT (fp8 DoubleRow, accum over d); gate
    pass streams the w13 DMA (up k=0,1 stream too: 8 open PSUM accumulation
    groups, one per 2KB bank); up pairs 1,2 re-read SBUF into the second
    half of the gate banks (sequential per bank).
  a^T = Silu(hT_gate) * hT_up * 2^-9, split hi+lo into TWO fp8 tensors
    (deq(hi)+deq(lo) carries ~8 mantissa bits) so mm2 runs fp8 DoubleRow.
    Silu on ACT, the scaled multiply on DVE, hi/lo casts on GPSIMD.
  mm2 ye[tok,d] = aT_hi/aT_lo x w2T, part-major over the 3 w2 DMA parts
    with MT*4 concurrently-open PSUM groups; PSUM->SBUF copies (ACT/DVE
    alternating, fused per-token comb*2^9 scale) then one bf16 DMA per
    128-token chunk.
"""

import sys

if "/opt/trn_rl_repo" not in sys.path:
    sys.path.insert(0, "/opt/trn_rl_repo")

import numpy as np
import ml_dtypes

T, D, I, E, TOPK, GROUP = 512, 2048, 768, 16, 8, 32
N_CORES = 8
E_LOC = E // N_CORES  # experts per core
S_A = 2.0 ** -9       # fixed pre-scale for fp8 hi/lo split of activations

FP8 = ml_dtypes.float8_e4m3      # TRN FP8_EXP4 (max 240) == bass dt.float8e4
BF16 = ml_dtypes.bfloat16

_FP4_TABLE = np.array(
    [0.0, 0.5, 1.0, 1.5, 2.0, 3.0, 4.0, 6.0,
     -0.0, -0.5, -1.0, -1.5, -2.0, -3.0, -4.0, -6.0], dtype=np.float32)


def _dequant_mxfp4(w_packed, sf):
    lo = _FP4_TABLE[w_packed & 0xF]
    hi = _FP4_TABLE[(w_packed >> 4) & 0xF]
    w = np.stack([lo, hi], axis=-1).reshape(*w_packed.shape[:-1], -1)
    s = (sf.astype(np.uint32) << 23).view(np.float32)
    w = w.reshape(*sf.shape, GROUP) * s[..., None]
    return w.reshape(*w_packed.shape[:-1], 2 * w_packed.shape[-1])


_PROGRAM_CACHE = {}


def _build_program(caps, split_waits=True):
    """caps: tuple of per-slot token capacities, each <= 256."""
    import concourse.bass as bass
    import concourse.mybir as mybir
    import concourse.tile as tile

    _TC = tile.TileContext

    def _split_excess_waits(nc):
        # This walrus build accepts only ONE sem-wait per instruction; hoist
        # extra waits onto standalone EventSemaphore (pure-wait) instructions
        # on the same engine, which execute in order ahead of the original.
        n = 0
        for f in nc.m.functions:
            for b in f.blocks:
                out = []
                for ins in b.instructions:
                    si = ins.sync_info
                    waits = list(si.on_wait) if (si and si.on_wait) else []
                    if len(waits) > 1:
                        for k, w in enumerate(waits[:-1]):
                            out.append(mybir.InstEventSemaphore(
                                name=f"{ins.name}-xw{k}", engine=ins.engine,
                                ins=[], outs=[],
                                sync_info=mybir.SyncInfo(
                                    on_wait=[w], on_update=[])))
                            n += 1
                        si.on_wait = waits[-1:]
                    out.append(ins)
                b.instructions = out
        return n

    dt = mybir.dt
    S = len(caps)
    assert all(c <= 256 for c in caps)
    DT, IT = D // 128, I // 128      # 16, 6
    SLOTS = sum(caps)
    OFF = [sum(caps[:s]) for s in range(S)]          # xgT slot offsets
    MTs = [-(-c // 128) for c in caps]               # chunks per slot
    TOTM = sum(MTs)
    W13P, W2P = 6, 3                 # DMA parts per slot weight
    FH, KH = 2 * I // W13P, IT // W2P  # 256 f-cols (one k-pair), 2 i-tiles
    AF = mybir.ActivationFunctionType

    nc = bass.Bass()
    xgt_d = nc.dram_tensor("xgt", [128, DT, SLOTS], dt.float8e4, kind="ExternalInput")
    # w13 parts are f-column-major (one k-pair per part: 3 gate pairs then 3
    # up pairs) so each pair's PSUM accumulation closes as its part lands.
    w13_d = nc.dram_tensor("w13t", [S, W13P, 128, DT, FH], dt.float8e4, kind="ExternalInput")
    w2_d = nc.dram_tensor("w2t", [S, IT, 128, D], dt.float8e4, kind="ExternalInput")
    comb_d = nc.dram_tensor("combg", [128, TOTM], dt.float32, kind="ExternalInput")
    ye_d = nc.dram_tensor("ye", [TOTM, 128, D], dt.bfloat16, kind="ExternalOutput")

    with _TC(nc) as tc:
        with (
            tc.tile_pool(name="inp", bufs=1) as inp,
            tc.tile_pool(name="wts", bufs=1) as wtsp,
            tc.tile_pool(name="xg", bufs=1) as xgp,
            tc.tile_pool(name="act", bufs=1) as actp,
            tc.tile_pool(name="at", bufs=1) as atp,
            tc.tile_pool(name="yout", bufs=1) as youtp,
            tc.tile_pool(name="ps_h", bufs=1, space="PSUM") as psh,
        ):
            # ---- DMAs in consumption order on the SP ring ----
            # comb rides the ACT queue so it never delays the SP stream.
            combg = inp.tile([128, TOTM], dt.float32, tag="cg")
            nc.scalar.dma_start(combg[:], comb_d[:])
            # xgT: host pre-gathers/transposes (it computes the routing
            # anyway), partition-major so the DMA is one big burst.
            xgT = xgp.tile([128, DT, SLOTS], dt.float8e4, tag="xgT")
            nc.sync.dma_start(xgT[:], xgt_d[:])
            # weights interleaved per slot (w13 s, w2 s) so each slot's mm2
            # runs as early as possible and only the last slot's mm2 + copies
            # sit in the tail behind the final w2 transfer.
            w13t = [[None] * W13P for _ in range(S)]
            w2t = [[None] * W2P for _ in range(S)]
            for s in range(S):
                for p in range(W13P):
                    wt = wtsp.tile([128, DT, FH], dt.float8e4, tag=f"w13_{s}_{p}")
                    nc.sync.dma_start(wt[:], w13_d[s, p])
                    w13t[s][p] = wt
                for p in range(W2P):
                    w2 = wtsp.tile([128, KH, D], dt.float8e4, tag=f"w2_{s}_{p}")
                    nc.sync.dma_start(
                        w2[:], w2_d[s, p * KH:(p + 1) * KH].rearrange("k p f -> p k f"))
                    w2t[s][p] = w2



            def mm1_pair(s, dst_pair, part):
                # One k-pair accumulation: contract all DT d-tiles of w13
                # part `part` against this slot's gathered tokens.  The two
                # k columns go to dst_pair kk=0,1 (separate PSUM banks).
                c = caps[s]
                for u in range(DT // 2):
                    for kk in range(2):
                        nc.tensor.matmul(
                            dst_pair[:, kk, 0:c],
                            w13t[s][part][:, 2 * u:2 * u + 2,
                                          kk * 128:(kk + 1) * 128],
                            xgT[:, 2 * u:2 * u + 2, OFF[s]:OFF[s] + caps[s]],
                            start=(u == 0), stop=(u == DT // 2 - 1),
                            perf_mode=mybir.MatmulPerfMode.DoubleRow)

            # Per slot: mm1 + activation chain streaming the w13 DMA (gate
            # pair v closes as part v lands -> Silu; up pair v closes as part
            # 3+v lands -> stt/cast/sub), then mm2 part-major with all 8
            # PSUM banks open so after the last w2 part lands only that
            # part's pass + copies remain; ye DMAs go last on the SP ring.
            yes = [youtp.tile([128, MTs[s], D], dt.bfloat16, tag=f"ye_{s}",
                              name=f"ye_{s}") for s in range(S)]
            mi = 0
            for s in range(S):
                c = caps[s]
                gps = [psh.tile([128, 2, 512], dt.float32, tag=f"hp{v}",
                                name=f"g{v}_{s}") for v in range(3)]
                # per-pair chain scratch so pairs don't false-serialize on
                # whole-tile dependencies
                sils = [actp.tile([128, 2, c], dt.float32, tag=f"sil{v}",
                                  name=f"sil{v}_{s}") for v in range(3)]
                as2s = [actp.tile([128, 2, c], dt.float32, tag=f"as2{v}",
                                  name=f"as2{v}_{s}") for v in range(3)]
                for v in range(3):
                    mm1_pair(s, gps[v], v)
                    nc.scalar.activation(sils[v][:], gps[v][:, :, 0:c],
                                         AF.Silu)
                # Per-pair aT tiles so mm2's part-p pass depends only on
                # chain pair p.  Width padded to full 128-token chunks: the
                # dual-fp8 Ldweights ISA check rejects partial-row loads, so
                # mm2 always loads [128, 2(stride 256), 128] like mm1.
                cpad = MTs[s] * 128
                aThi = [atp.tile([128, 2, cpad], dt.float8e4, tag=f"aThi_{s}_{v}",
                                 name=f"aThi_{s}_{v}") for v in range(3)]
                aTlo = [atp.tile([128, 2, cpad], dt.float8e4, tag=f"aTlo_{s}_{v}",
                                 name=f"aTlo_{s}_{v}") for v in range(3)]
                for v in range(3):
                    ups = psh.tile([128, 2, 512], dt.float32, tag=f"hp{v}",
                                   name=f"u{v}_{s}")
                    mm1_pair(s, ups, 3 + v)
                    # stt's stay on DVE so they stream with the parts; the
                    # hi/lo split runs on Pool for pairs 0,1 and on DVE for
                    # the critical last pair (no cross-engine hop before mm2)
                    nc.vector.scalar_tensor_tensor(
                        as2s[v][:], sils[v][:], S_A, ups[:, :, 0:c],
                        op0=mybir.AluOpType.mult, op1=mybir.AluOpType.mult)
                    eng = nc.vector if v == 2 else nc.gpsimd
                    eng.tensor_copy(aThi[v][:, :, 0:c], as2s[v][:])
                    eng.tensor_tensor(
                        aTlo[v][:, :, 0:c], as2s[v][:], aThi[v][:, :, 0:c],
                        op=mybir.AluOpType.subtract)
                # all 8 PSUM banks as 4 pair-tiles: group (m, dq) lives in
                # pair 2*m + dq//2, bank dq%2 — so the PSUM->SBUF copies can
                # be 1024-wide pair reads (half the copies and sem hops).
                # Tokens beyond c are garbage fp8; their PSUM rows are never
                # copied out.
                yhp = [psh.tile([128, 2, 512], dt.float32, tag=f"hp{p}",
                                name=f"yhp{p}_{s}") for p in range(4)]
                for p in range(W2P):
                    for m in range(MTs[s]):
                        for dq in range(4):
                            yh = yhp[2 * m + dq // 2][:, dq % 2, :]
                            for at in (aThi[p], aTlo[p]):
                                nc.tensor.matmul(
                                    yh, at[:, :, m * 128:(m + 1) * 128],
                                    w2t[s][p][:, 0:2, dq * 512:(dq + 1) * 512],
                                    start=(p == 0 and at is aThi[p]),
                                    stop=(p == W2P - 1 and at is aTlo[p]),
                                    perf_mode=mybir.MatmulPerfMode.DoubleRow)
                # 1024-wide pair copies on ACT/DVE (GPSIMD cannot access
                # PSUM), m0-first so each ye chunk DMA fires earliest
                for pi in range(2 * MTs[s]):
                    m, dqp = pi // 2, pi % 2
                    rows = min(128, caps[s] - 128 * m)
                    cg = combg[0:rows, mi + m:mi + m + 1]
                    dst = yes[s][0:rows, m, dqp * 1024:(dqp + 1) * 1024]
                    src = yhp[2 * m + dqp][0:rows]
                    if pi % 2 == 0:
                        nc.scalar.activation(dst, src, AF.Copy, scale=cg)
                    else:
                        nc.vector.tensor_scalar(dst, src, cg, None,
                                                op0=mybir.AluOpType.mult)
                for m in range(MTs[s]):
                    rows = min(128, caps[s] - 128 * m)
                    nc.sync.dma_start(ye_d[mi + m, 0:rows, :],
                                      yes[s][0:rows, m, :])
                mi += MTs[s]

    nc.finalize()
    if split_waits:
        _split_excess_waits(nc)
    return nc


def kernel(hidden_states, topk_weights, topk_ids, w13_weight, w13_weight_scale,
           w2_weight, w2_weight_scale):
    from concourse.bass_utils import run_bass_kernel_spmd

    x = np.asarray(hidden_states, dtype=np.float32)
    tw = np.asarray(topk_weights, dtype=np.float32)
    ti = np.asarray(topk_ids)

    # host routing: combine weights + per-expert token lists
    comb = np.zeros((T, E), np.float32)
    for k in range(TOPK):
        np.add.at(comb, (np.arange(T), ti[:, k]), tw[:, k])
    routed = comb > 0.0
    idx = [np.nonzero(routed[:, e])[0] for e in range(E)]

    # per-core slots: each slot is (expert, token subrange) with <= 256
    # tokens; slot 0 gets the larger share so caps stay tight.
    core_slots = []
    for core in range(N_CORES):
        slots = []
        for le in range(E_LOC):
            e = core * E_LOC + le
            n = len(idx[e])
            for st in range(0, max(n, 1), 256):
                slots.append((e, st, min(256, n - st)))
        slots.sort(key=lambda t: -t[2])
        core_slots.append(slots)
    NS = max(len(s) for s in core_slots)
    for slots in core_slots:
        while len(slots) < NS:
            slots.append((0, 0, 0))
    caps = tuple(
        max(16, -(-max(core_slots[c][s][2] for c in range(N_CORES)) // 8) * 8)
        for s in range(NS))

    if caps not in _PROGRAM_CACHE:
        _PROGRAM_CACHE[caps] = _build_program(caps)
    nc = _PROGRAM_CACHE[caps]

    SLOTS = sum(caps)
    OFF = [sum(caps[:s]) for s in range(NS)]
    MTs = [-(-c // 128) for c in caps]
    TOTM = sum(MTs)
    MOFF = [sum(MTs[:s]) for s in range(NS)]
    DT, IT = D // 128, I // 128

    # host staging quantization + gather/transpose + lossless weight conv
    xq8 = x.astype(FP8)
    w13 = _dequant_mxfp4(np.asarray(w13_weight), np.asarray(w13_weight_scale))
    w2 = _dequant_mxfp4(np.asarray(w2_weight), np.asarray(w2_weight_scale))
    # w13 parts are f-column-major k-pairs: part j = cols [256j, 256j+256)
    # of w13[e].T, laid out [128, DT, 256] partition-major
    W13P, FH = 6, 256
    w13t8 = [np.ascontiguousarray(
        w13[e].T.astype(FP8).reshape(DT, 128, W13P, FH).transpose(2, 1, 0, 3))
        for e in range(E)]
    w2t8 = [w2[e].T.astype(FP8).reshape(IT, 128, D) for e in range(E)]

    in_maps = []
    for core in range(N_CORES):
        m = {}
        xgt = np.zeros((128, DT, SLOTS), FP8)
        cg = np.zeros((128, TOTM), np.float32)
        w13m = np.zeros((NS, W13P, 128, DT, FH), FP8)
        w2m = np.zeros((NS, IT, 128, D), FP8)
        for s, (e, st, n) in enumerate(core_slots[core]):
            if n == 0:
                continue
            ix = idx[e][st:st + n]
            xgt[:, :, OFF[s]:OFF[s] + n] = np.transpose(
                xq8[ix].T.reshape(DT, 128, n), (1, 0, 2))
            cw = comb[ix, e] / S_A      # undo the fp8 pre-scale
            for mm in range(MTs[s]):
                r = min(128, n - 128 * mm)
                if r > 0:
                    cg[0:r, MOFF[s] + mm] = cw[128 * mm:128 * mm + r]
            w13m[s] = w13t8[e]
            w2m[s] = w2t8[e]
        m["xgt"] = xgt
        m["combg"] = cg
        m["w13t"] = w13m
        m["w2t"] = w2m
        in_maps.append(m)

    res = run_bass_kernel_spmd(nc, in_maps, list(range(N_CORES)))

    out = np.zeros((T, D), np.float32)
    for core in range(N_CORES):
        ye = np.asarray(res.results[core]["ye"], dtype=np.float32)
        for s, (e, st, n) in enumerate(core_slots[core]):
            if n == 0:
                continue
            ix = idx[e][st:st + n]
            for mm in range(MTs[s]):
                r = min(128, n - 128 * mm)
                if r > 0:
                    out[ix[128 * mm:128 * mm + r]] += ye[MOFF[s] + mm, 0:r]
    return out


# revision 23
# speedup vs baseline: 1.0598x; 1.0067x over previous
"""DeepseekV4 Mega-MoE experts layer on 8 Trainium2 NeuronCores.

Strategy (expert-parallel, per sharding hint):
  - 16 experts sharded 2-per-core across 8 cores; each core receives its two
    experts' weights (losslessly converted: mxfp4*ue8m0 dequant values are
    exactly representable in TRN fp8_e4m3 for both w13 and w2).
  - Staging fp8 quantization of hidden_states runs on the host (direct
    fp32->fp8e4 cast, 1/4 the DMA bytes of fp32); the host also gathers
    tokens per expert (the "all-to-all") and sums per-expert outputs (the
    "combine").

The kernel is HBM-DMA bound (12MB/core at 360 B/ns ~= 33us of DMA busy vs
~20us PE).  v2 scheduling goals:
  - slot caps sized to the actual max token counts (rounded /16), not 256,
    cutting xgT and ye bytes;
  - one gapless DMA stream on the SP queue: comb+xgT, w13 slot0, w13 slot1,
    w2 slot0, w2 slot1, then every ye output chunk.  Outputs are emitted
    last so input transfers never wait on compute;
  - mm2 runs part-major with all 8 PSUM banks open (yh tiles alias mm1's
    hp/up banks), so after the last w2 part lands only that part's matmul
    pass (~1.7us) + copies remain before the final ye chunks stream out.

Per-core device pipeline:
  mm1 hT[f,tok] = w13T chunks x xgT (fp8 DoubleRow, accum over d); gate
    pass streams the w13 DMA (up k=0,1 stream too: 8 open PSUM accumulation
    groups, one per 2KB bank); up pairs 1,2 re-read SBUF into the second
    half of the gate banks (sequential per bank).
  a^T = Silu(hT_gate) * hT_up * 2^-9, split hi+lo into TWO fp8 tensors
    (deq(hi)+deq(lo) carries ~8 mantissa bits) so mm2 runs fp8 DoubleRow.
    Silu on ACT, the scaled multiply on DVE, hi/lo casts on GPSIMD.
  mm2 ye[tok,d] = aT_hi/aT_lo x w2T, part-major over the 3 w2 DMA parts
    with MT*4 concurrently-open PSUM groups; PSUM->SBUF copies (ACT/DVE
    alternating, fused per-token comb*2^9 scale) then one bf16 DMA per
    128-token chunk.
"""

import sys

if "/opt/trn_rl_repo" not in sys.path:
    sys.path.insert(0, "/opt/trn_rl_repo")

import numpy as np
import ml_dtypes

T, D, I, E, TOPK, GROUP = 512, 2048, 768, 16, 8, 32
N_CORES = 8
E_LOC = E // N_CORES  # experts per core
S_A = 2.0 ** -9       # fixed pre-scale for fp8 hi/lo split of activations

FP8 = ml_dtypes.float8_e4m3      # TRN FP8_EXP4 (max 240) == bass dt.float8e4
BF16 = ml_dtypes.bfloat16

_FP4_TABLE = np.array(
    [0.0, 0.5, 1.0, 1.5, 2.0, 3.0, 4.0, 6.0,
     -0.0, -0.5, -1.0, -1.5, -2.0, -3.0, -4.0, -6.0], dtype=np.float32)


def _dequant_mxfp4(w_packed, sf):
    lo = _FP4_TABLE[w_packed & 0xF]
    hi = _FP4_TABLE[(w_packed >> 4) & 0xF]
    w = np.stack([lo, hi], axis=-1).reshape(*w_packed.shape[:-1], -1)
    s = (sf.astype(np.uint32) << 23).view(np.float32)
    w = w.reshape(*sf.shape, GROUP) * s[..., None]
    return w.reshape(*w_packed.shape[:-1], 2 * w_packed.shape[-1])


_PROGRAM_CACHE = {}


def _build_program(caps, split_waits=True):
    """caps: tuple of per-slot token capacities, each <= 256."""
    import concourse.bass as bass
    import concourse.mybir as mybir
    import concourse.tile as tile

    _TC = tile.TileContext

    def _split_excess_waits(nc):
        # This walrus build accepts only ONE sem-wait per instruction; hoist
        # extra waits onto standalone EventSemaphore (pure-wait) instructions
        # on the same engine, which execute in order ahead of the original.
        n = 0
        for f in nc.m.functions:
            for b in f.blocks:
                out = []
                for ins in b.instructions:
                    si = ins.sync_info
                    waits = list(si.on_wait) if (si and si.on_wait) else []
                    if len(waits) > 1:
                        for k, w in enumerate(waits[:-1]):
                            out.append(mybir.InstEventSemaphore(
                                name=f"{ins.name}-xw{k}", engine=ins.engine,
                                ins=[], outs=[],
                                sync_info=mybir.SyncInfo(
                                    on_wait=[w], on_update=[])))
                            n += 1
                        si.on_wait = waits[-1:]
                    out.append(ins)
                b.instructions = out
        return n

    dt = mybir.dt
    S = len(caps)
    assert all(c <= 256 for c in caps)
    DT, IT = D // 128, I // 128      # 16, 6
    SLOTS = sum(caps)
    OFF = [sum(caps[:s]) for s in range(S)]          # xgT slot offsets
    MTs = [-(-c // 128) for c in caps]               # chunks per slot
    TOTM = sum(MTs)
    W13P, W2P = 6, 3                 # DMA parts per slot weight
    FH, KH = 2 * I // W13P, IT // W2P  # 256 f-cols (one k-pair), 2 i-tiles
    AF = mybir.ActivationFunctionType

    nc = bass.Bass()
    xgt_d = nc.dram_tensor("xgt", [128, DT, SLOTS], dt.float8e4, kind="ExternalInput")
    # w13 parts are f-column-major (one k-pair per part: 3 gate pairs then 3
    # up pairs) so each pair's PSUM accumulation closes as its part lands.
    w13_d = nc.dram_tensor("w13t", [S, W13P, 128, DT, FH], dt.float8e4, kind="ExternalInput")
    w2_d = nc.dram_tensor("w2t", [S, IT, 128, D], dt.float8e4, kind="ExternalInput")
    comb_d = nc.dram_tensor("combg", [128, TOTM], dt.float32, kind="ExternalInput")
    ye_d = nc.dram_tensor("ye", [TOTM, 128, D], dt.bfloat16, kind="ExternalOutput")

    with _TC(nc) as tc:
        with (
            tc.tile_pool(name="inp", bufs=1) as inp,
            tc.tile_pool(name="wts", bufs=1) as wtsp,
            tc.tile_pool(name="xg", bufs=1) as xgp,
            tc.tile_pool(name="act", bufs=1) as actp,
            tc.tile_pool(name="at", bufs=1) as atp,
            tc.tile_pool(name="yout", bufs=1) as youtp,
            tc.tile_pool(name="ps_h", bufs=1, space="PSUM") as psh,
        ):
            # ---- DMAs in consumption order on the SP ring ----
            # comb rides the ACT queue so it never delays the SP stream.
            combg = inp.tile([128, TOTM], dt.float32, tag="cg")
            nc.scalar.dma_start(combg[:], comb_d[:])
            # xgT: host pre-gathers/transposes (it computes the routing
            # anyway), partition-major so the DMA is one big burst.
            xgT = xgp.tile([128, DT, SLOTS], dt.float8e4, tag="xgT")
            nc.sync.dma_start(xgT[:], xgt_d[:])
            # weights interleaved per slot (w13 s, w2 s) so each slot's mm2
            # runs as early as possible and only the last slot's mm2 + copies
            # sit in the tail behind the final w2 transfer.
            w13t = [[None] * W13P for _ in range(S)]
            w2t = [[None] * W2P for _ in range(S)]
            for s in range(S):
                for p in range(W13P):
                    wt = wtsp.tile([128, DT, FH], dt.float8e4, tag=f"w13_{s}_{p}")
                    nc.sync.dma_start(wt[:], w13_d[s, p])
                    w13t[s][p] = wt
                for p in range(W2P):
                    w2 = wtsp.tile([128, KH, D], dt.float8e4, tag=f"w2_{s}_{p}")
                    nc.sync.dma_start(
                        w2[:], w2_d[s, p * KH:(p + 1) * KH].rearrange("k p f -> p k f"))
                    w2t[s][p] = w2



            def mm1_pair(s, dst_pair, part):
                # One k-pair accumulation: contract all DT d-tiles of w13
                # part `part` against this slot's gathered tokens.  The two
                # k columns go to dst_pair kk=0,1 (separate PSUM banks).
                c = caps[s]
                for u in range(DT // 2):
                    for kk in range(2):
                        nc.tensor.matmul(
                            dst_pair[:, kk, 0:c],
                            w13t[s][part][:, 2 * u:2 * u + 2,
                                          kk * 128:(kk + 1) * 128],
                            xgT[:, 2 * u:2 * u + 2, OFF[s]:OFF[s] + caps[s]],
                            start=(u == 0), stop=(u == DT // 2 - 1),
                            perf_mode=mybir.MatmulPerfMode.DoubleRow)

            # Per slot: mm1 + activation chain streaming the w13 DMA (gate
            # pair v closes as part v lands -> Silu; up pair v closes as part
            # 3+v lands -> stt/cast/sub), then mm2 part-major with all 8
            # PSUM banks open so after the last w2 part lands only that
            # part's pass + copies remain; ye DMAs go last on the SP ring.
            yes = [youtp.tile([128, MTs[s], D], dt.bfloat16, tag=f"ye_{s}",
                              name=f"ye_{s}") for s in range(S)]
            mi = 0
            for s in range(S):
                c = caps[s]
                gps = [psh.tile([128, 2, 512], dt.float32, tag=f"hp{v}",
                                name=f"g{v}_{s}") for v in range(3)]
                # per-pair chain scratch so pairs don't false-serialize on
                # whole-tile dependencies
                sils = [actp.tile([128, 2, c], dt.float32, tag=f"sil{v}",
                                  name=f"sil{v}_{s}") for v in range(3)]
                as2s = [actp.tile([128, 2, c], dt.float32, tag=f"as2{v}",
                                  name=f"as2{v}_{s}") for v in range(3)]
                for v in range(3):
                    mm1_pair(s, gps[v], v)
                    nc.scalar.activation(sils[v][:], gps[v][:, :, 0:c],
                                         AF.Silu)
                # Per-pair aT tiles so mm2's part-p pass depends only on
                # chain pair p.  Width padded to full 128-token chunks: the
                # dual-fp8 Ldweights ISA check rejects partial-row loads, so
                # mm2 always loads [128, 2(stride 256), 128] like mm1.
                cpad = MTs[s] * 128
                aThi = [atp.tile([128, 2, cpad], dt.float8e4, tag=f"aThi_{s}_{v}",
                                 name=f"aThi_{s}_{v}") for v in range(3)]
                aTlo = [atp.tile([128, 2, cpad], dt.float8e4, tag=f"aTlo_{s}_{v}",
                                 name=f"aTlo_{s}_{v}") for v in range(3)]
                for v in range(3):
                    ups = psh.tile([128, 2, 512], dt.float32, tag=f"hp{v}",
                                   name=f"u{v}_{s}")
                    mm1_pair(s, ups, 3 + v)
                    # stt's stay on DVE so they stream with the parts; the
                    # hi/lo split runs on Pool for pairs 0,1 and on DVE for
                    # the critical last pair (no cross-engine hop before mm2)
                    nc.vector.scalar_tensor_tensor(
                        as2s[v][:], sils[v][:], S_A, ups[:, :, 0:c],
                        op0=mybir.AluOpType.mult, op1=mybir.AluOpType.mult)
                    eng = nc.vector if v == 2 else nc.gpsimd
                    eng.tensor_copy(aThi[v][:, :, 0:c], as2s[v][:])
                    eng.tensor_tensor(
                        aTlo[v][:, :, 0:c], as2s[v][:], aThi[v][:, :, 0:c],
                        op=mybir.AluOpType.subtract)
                # all 8 PSUM banks as 4 pair-tiles: group (m, dq) lives in
                # pair 2*m + dq//2, bank dq%2 — so the PSUM->SBUF copies can
                # be 1024-wide pair reads (half the copies and sem hops).
                # Tokens beyond c are garbage fp8; their PSUM rows are never
                # copied out.
                yhp = [psh.tile([128, 2, 512], dt.float32, tag=f"hp{p}",
                                name=f"yhp{p}_{s}") for p in range(4)]
                for p in range(W2P):
                    for m in range(MTs[s]):
                        for dq in range(4):
                            yh = yhp[2 * m + dq // 2][:, dq % 2, :]
                            for at in (aThi[p], aTlo[p]):
                                nc.tensor.matmul(
                                    yh, at[:, :, m * 128:(m + 1) * 128],
                                    w2t[s][p][:, 0:2, dq * 512:(dq + 1) * 512],
                                    start=(p == 0 and at is aThi[p]),
                                    stop=(p == W2P - 1 and at is aTlo[p]),
                                    perf_mode=mybir.MatmulPerfMode.DoubleRow)
                # 1024-wide pair copies on ACT/DVE (GPSIMD cannot access
                # PSUM), m0-first so each ye chunk DMA fires earliest
                for pi in range(2 * MTs[s]):
                    m, dqp = pi // 2, pi % 2
                    rows = min(128, caps[s] - 128 * m)
                    cg = combg[0:rows, mi + m:mi + m + 1]
                    dst = yes[s][0:rows, m, dqp * 1024:(dqp + 1) * 1024]
                    src = yhp[2 * m + dqp][0:rows]
                    if pi % 2 == 0:
                        nc.scalar.activation(dst, src, AF.Copy, scale=cg)
                    else:
                        nc.vector.tensor_scalar(dst, src, cg, None,
                                                op0=mybir.AluOpType.mult)
                for m in range(MTs[s]):
                    rows = min(128, caps[s] - 128 * m)
                    if s == S - 1 and m == 0:
                        # tail chunk: two half-D transfers on idle queues so
                        # each half waits only its own copy and the two
                        # DMA-issue latencies overlap instead of serializing
                        # behind the blocked SP ring
                        nc.scalar.dma_start(ye_d[mi + m, 0:rows, 0:1024],
                                            yes[s][0:rows, m, 0:1024])
                        nc.gpsimd.dma_start(ye_d[mi + m, 0:rows, 1024:2048],
                                            yes[s][0:rows, m, 1024:2048])
                    else:
                        nc.sync.dma_start(ye_d[mi + m, 0:rows, :],
                                          yes[s][0:rows, m, :])
                mi += MTs[s]

    nc.finalize()
    if split_waits:
        _split_excess_waits(nc)
    return nc


def kernel(hidden_states, topk_weights, topk_ids, w13_weight, w13_weight_scale,
           w2_weight, w2_weight_scale):
    from concourse.bass_utils import run_bass_kernel_spmd

    x = np.asarray(hidden_states, dtype=np.float32)
    tw = np.asarray(topk_weights, dtype=np.float32)
    ti = np.asarray(topk_ids)

    # host routing: combine weights + per-expert token lists
    comb = np.zeros((T, E), np.float32)
    for k in range(TOPK):
        np.add.at(comb, (np.arange(T), ti[:, k]), tw[:, k])
    routed = comb > 0.0
    idx = [np.nonzero(routed[:, e])[0] for e in range(E)]

    # per-core slots: each slot is (expert, token subrange) with <= 256
    # tokens; slot 0 gets the larger share so caps stay tight.
    core_slots = []
    for core in range(N_CORES):
        slots = []
        for le in range(E_LOC):
            e = core * E_LOC + le
            n = len(idx[e])
            for st in range(0, max(n, 1), 256):
                slots.append((e, st, min(256, n - st)))
        slots.sort(key=lambda t: -t[2])
        core_slots.append(slots)
    NS = max(len(s) for s in core_slots)
    for slots in core_slots:
        while len(slots) < NS:
            slots.append((0, 0, 0))
    caps = tuple(
        max(16, -(-max(core_slots[c][s][2] for c in range(N_CORES)) // 8) * 8)
        for s in range(NS))

    if caps not in _PROGRAM_CACHE:
        _PROGRAM_CACHE[caps] = _build_program(caps)
    nc = _PROGRAM_CACHE[caps]

    SLOTS = sum(caps)
    OFF = [sum(caps[:s]) for s in range(NS)]
    MTs = [-(-c // 128) for c in caps]
    TOTM = sum(MTs)
    MOFF = [sum(MTs[:s]) for s in range(NS)]
    DT, IT = D // 128, I // 128

    # host staging quantization + gather/transpose + lossless weight conv
    xq8 = x.astype(FP8)
    w13 = _dequant_mxfp4(np.asarray(w13_weight), np.asarray(w13_weight_scale))
    w2 = _dequant_mxfp4(np.asarray(w2_weight), np.asarray(w2_weight_scale))
    # w13 parts are f-column-major k-pairs: part j = cols [256j, 256j+256)
    # of w13[e].T, laid out [128, DT, 256] partition-major
    W13P, FH = 6, 256
    w13t8 = [np.ascontiguousarray(
        w13[e].T.astype(FP8).reshape(DT, 128, W13P, FH).transpose(2, 1, 0, 3))
        for e in range(E)]
    w2t8 = [w2[e].T.astype(FP8).reshape(IT, 128, D) for e in range(E)]

    in_maps = []
    for core in range(N_CORES):
        m = {}
        xgt = np.zeros((128, DT, SLOTS), FP8)
        cg = np.zeros((128, TOTM), np.float32)
        w13m = np.zeros((NS, W13P, 128, DT, FH), FP8)
        w2m = np.zeros((NS, IT, 128, D), FP8)
        for s, (e, st, n) in enumerate(core_slots[core]):
            if n == 0:
                continue
            ix = idx[e][st:st + n]
            xgt[:, :, OFF[s]:OFF[s] + n] = np.transpose(
                xq8[ix].T.reshape(DT, 128, n), (1, 0, 2))
            cw = comb[ix, e] / S_A      # undo the fp8 pre-scale
            for mm in range(MTs[s]):
                r = min(128, n - 128 * mm)
                if r > 0:
                    cg[0:r, MOFF[s] + mm] = cw[128 * mm:128 * mm + r]
            w13m[s] = w13t8[e]
            w2m[s] = w2t8[e]
        m["xgt"] = xgt
        m["combg"] = cg
        m["w13t"] = w13m
        m["w2t"] = w2m
        in_maps.append(m)

    res = run_bass_kernel_spmd(nc, in_maps, list(range(N_CORES)))

    out = np.zeros((T, D), np.float32)
    for core in range(N_CORES):
        ye = np.asarray(res.results[core]["ye"], dtype=np.float32)
        for s, (e, st, n) in enumerate(core_slots[core]):
            if n == 0:
                continue
            ix = idx[e][st:st + n]
            for mm in range(MTs[s]):
                r = min(128, n - 128 * mm)
                if r > 0:
                    out[ix[128 * mm:128 * mm + r]] += ye[MOFF[s] + mm, 0:r]
    return out


# revision 31
# speedup vs baseline: 1.0602x; 1.0003x over previous
"""DeepseekV4 Mega-MoE experts layer on 8 Trainium2 NeuronCores.

Strategy (expert-parallel, per sharding hint):
  - 16 experts sharded 2-per-core across 8 cores; each core receives its two
    experts' weights (losslessly converted: mxfp4*ue8m0 dequant values are
    exactly representable in TRN fp8_e4m3 for both w13 and w2).
  - Staging fp8 quantization of hidden_states runs on the host (direct
    fp32->fp8e4 cast, 1/4 the DMA bytes of fp32); the host also gathers
    tokens per expert (the "all-to-all") and sums per-expert outputs (the
    "combine").

The kernel is HBM-DMA bound (12MB/core at 360 B/ns ~= 33us of DMA busy vs
~20us PE).  v2 scheduling goals:
  - slot caps sized to the actual max token counts (rounded /16), not 256,
    cutting xgT and ye bytes;
  - one gapless DMA stream on the SP queue: comb+xgT, w13 slot0, w13 slot1,
    w2 slot0, w2 slot1, then every ye output chunk.  Outputs are emitted
    last so input transfers never wait on compute;
  - mm2 runs part-major with all 8 PSUM banks open (yh tiles alias mm1's
    hp/up banks), so after the last w2 part lands only that part's matmul
    pass (~1.7us) + copies remain before the final ye chunks stream out.

Per-core device pipeline:
  mm1 hT[f,tok] = w13T chunks x xgT (fp8 DoubleRow, accum over d); gate
    pass streams the w13 DMA (up k=0,1 stream too: 8 open PSUM accumulation
    groups, one per 2KB bank); up pairs 1,2 re-read SBUF into the second
    half of the gate banks (sequential per bank).
  a^T = Silu(hT_gate) * hT_up * 2^-9, split hi+lo into TWO fp8 tensors
    (deq(hi)+deq(lo) carries ~8 mantissa bits) so mm2 runs fp8 DoubleRow.
    Silu on ACT, the scaled multiply on DVE, hi/lo casts on GPSIMD.
  mm2 ye[tok,d] = aT_hi/aT_lo x w2T, part-major over the 3 w2 DMA parts
    with MT*4 concurrently-open PSUM groups; PSUM->SBUF copies (ACT/DVE
    alternating, fused per-token comb*2^9 scale) then one bf16 DMA per
    128-token chunk.
"""

import os
import sys

if "/opt/trn_rl_repo" not in sys.path:
    sys.path.insert(0, "/opt/trn_rl_repo")

# recover cleanly if a previous process left the NeuronCores wedged
os.environ.setdefault("NEURON_RT_RESET_CORES", "1")

import numpy as np
import ml_dtypes

T, D, I, E, TOPK, GROUP = 512, 2048, 768, 16, 8, 32
N_CORES = 8
E_LOC = E // N_CORES  # experts per core
S_A = 2.0 ** -9       # fixed pre-scale for fp8 hi/lo split of activations

FP8 = ml_dtypes.float8_e4m3      # TRN FP8_EXP4 (max 240) == bass dt.float8e4
BF16 = ml_dtypes.bfloat16

_FP4_TABLE = np.array(
    [0.0, 0.5, 1.0, 1.5, 2.0, 3.0, 4.0, 6.0,
     -0.0, -0.5, -1.0, -1.5, -2.0, -3.0, -4.0, -6.0], dtype=np.float32)


def _dequant_mxfp4(w_packed, sf):
    lo = _FP4_TABLE[w_packed & 0xF]
    hi = _FP4_TABLE[(w_packed >> 4) & 0xF]
    w = np.stack([lo, hi], axis=-1).reshape(*w_packed.shape[:-1], -1)
    s = (sf.astype(np.uint32) << 23).view(np.float32)
    w = w.reshape(*sf.shape, GROUP) * s[..., None]
    return w.reshape(*w_packed.shape[:-1], 2 * w_packed.shape[-1])


_PROGRAM_CACHE = {}

# Tail-slot output routing (picked by TimelineSim sweep): per half-D
# transfer (m0a, m0b, m1a, m1b) the issuing queue, and per chunk whether to
# merge its two halves into one full-D transfer.
_TAIL_QUEUES = ("act", "pool", "sp", "sp")
_TAIL_MERGE = (False, True)


def _build_program(caps, split_waits=True):
    """caps: tuple of per-slot token capacities, each <= 256."""
    import concourse.bass as bass
    import concourse.mybir as mybir
    import concourse.tile as tile

    _TC = tile.TileContext

    def _split_excess_waits(nc):
        # This walrus build accepts only ONE sem-wait per instruction; hoist
        # extra waits onto standalone EventSemaphore (pure-wait) instructions
        # on the same engine, which execute in order ahead of the original.
        n = 0
        for f in nc.m.functions:
            for b in f.blocks:
                out = []
                for ins in b.instructions:
                    si = ins.sync_info
                    waits = list(si.on_wait) if (si and si.on_wait) else []
                    if len(waits) > 1:
                        for k, w in enumerate(waits[:-1]):
                            out.append(mybir.InstEventSemaphore(
                                name=f"{ins.name}-xw{k}", engine=ins.engine,
                                ins=[], outs=[],
                                sync_info=mybir.SyncInfo(
                                    on_wait=[w], on_update=[])))
                            n += 1
                        si.on_wait = waits[-1:]
                    out.append(ins)
                b.instructions = out
        return n

    dt = mybir.dt
    S = len(caps)
    assert all(c <= 256 for c in caps)
    DT, IT = D // 128, I // 128      # 16, 6
    SLOTS = sum(caps)
    OFF = [sum(caps[:s]) for s in range(S)]          # xgT slot offsets
    MTs = [-(-c // 128) for c in caps]               # chunks per slot
    TOTM = sum(MTs)
    W13P, W2P = 6, 3                 # DMA parts per slot weight
    FH, KH = 2 * I // W13P, IT // W2P  # 256 f-cols (one k-pair), 2 i-tiles
    AF = mybir.ActivationFunctionType

    nc = bass.Bass()
    xgt_d = nc.dram_tensor("xgt", [128, DT, SLOTS], dt.float8e4, kind="ExternalInput")
    # w13 parts are f-column-major (one k-pair per part: 3 gate pairs then 3
    # up pairs) so each pair's PSUM accumulation closes as its part lands.
    w13_d = nc.dram_tensor("w13t", [S, W13P, 128, DT, FH], dt.float8e4, kind="ExternalInput")
    w2_d = nc.dram_tensor("w2t", [S, IT, 128, D], dt.float8e4, kind="ExternalInput")
    comb_d = nc.dram_tensor("combg", [128, TOTM], dt.float32, kind="ExternalInput")
    ye_d = nc.dram_tensor("ye", [TOTM, 128, D], dt.bfloat16, kind="ExternalOutput")

    with _TC(nc) as tc:
        with (
            tc.tile_pool(name="inp", bufs=1) as inp,
            tc.tile_pool(name="wts", bufs=1) as wtsp,
            tc.tile_pool(name="xg", bufs=1) as xgp,
            tc.tile_pool(name="act", bufs=1) as actp,
            tc.tile_pool(name="at", bufs=1) as atp,
            tc.tile_pool(name="yout", bufs=1) as youtp,
            tc.tile_pool(name="ps_h", bufs=1, space="PSUM") as psh,
        ):
            # ---- DMAs in consumption order on the SP ring ----
            # comb rides the ACT queue so it never delays the SP stream.
            combg = inp.tile([128, TOTM], dt.float32, tag="cg")
            nc.scalar.dma_start(combg[:], comb_d[:])
            # xgT: host pre-gathers/transposes (it computes the routing
            # anyway), partition-major so the DMA is one big burst.
            xgT = xgp.tile([128, DT, SLOTS], dt.float8e4, tag="xgT")
            nc.sync.dma_start(xgT[:], xgt_d[:])
            # weights interleaved per slot (w13 s, w2 s) so each slot's mm2
            # runs as early as possible and only the last slot's mm2 + copies
            # sit in the tail behind the final w2 transfer.
            w13t = [[None] * W13P for _ in range(S)]
            w2t = [[None] * W2P for _ in range(S)]
            for s in range(S):
                for p in range(W13P):
                    wt = wtsp.tile([128, DT, FH], dt.float8e4, tag=f"w13_{s}_{p}")
                    nc.sync.dma_start(wt[:], w13_d[s, p])
                    w13t[s][p] = wt
                for p in range(W2P):
                    w2 = wtsp.tile([128, KH, D], dt.float8e4, tag=f"w2_{s}_{p}")
                    nc.sync.dma_start(
                        w2[:], w2_d[s, p * KH:(p + 1) * KH].rearrange("k p f -> p k f"))
                    w2t[s][p] = w2



            def mm1_pair(s, dst_pair, part):
                # One k-pair accumulation: contract all DT d-tiles of w13
                # part `part` against this slot's gathered tokens.  The two
                # k columns go to dst_pair kk=0,1 (separate PSUM banks).
                c = caps[s]
                for u in range(DT // 2):
                    for kk in range(2):
                        nc.tensor.matmul(
                            dst_pair[:, kk, 0:c],
                            w13t[s][part][:, 2 * u:2 * u + 2,
                                          kk * 128:(kk + 1) * 128],
                            xgT[:, 2 * u:2 * u + 2, OFF[s]:OFF[s] + caps[s]],
                            start=(u == 0), stop=(u == DT // 2 - 1),
                            perf_mode=mybir.MatmulPerfMode.DoubleRow)

            # Per slot: mm1 + activation chain streaming the w13 DMA (gate
            # pair v closes as part v lands -> Silu; up pair v closes as part
            # 3+v lands -> stt/cast/sub), then mm2 part-major with all 8
            # PSUM banks open so after the last w2 part lands only that
            # part's pass + copies remain; ye DMAs go last on the SP ring.
            yes = [youtp.tile([128, MTs[s], D], dt.bfloat16, tag=f"ye_{s}",
                              name=f"ye_{s}") for s in range(S)]
            mi = 0
            for s in range(S):
                c = caps[s]
                gps = [psh.tile([128, 2, 512], dt.float32, tag=f"hp{v}",
                                name=f"g{v}_{s}") for v in range(3)]
                # per-pair chain scratch so pairs don't false-serialize on
                # whole-tile dependencies
                sils = [actp.tile([128, 2, c], dt.float32, tag=f"sil{v}",
                                  name=f"sil{v}_{s}") for v in range(3)]
                as2s = [actp.tile([128, 2, c], dt.float32, tag=f"as2{v}",
                                  name=f"as2{v}_{s}") for v in range(3)]
                for v in range(3):
                    mm1_pair(s, gps[v], v)
                    nc.scalar.activation(sils[v][:], gps[v][:, :, 0:c],
                                         AF.Silu)
                # Per-pair aT tiles so mm2's part-p pass depends only on
                # chain pair p.  Width padded to full 128-token chunks: the
                # dual-fp8 Ldweights ISA check rejects partial-row loads, so
                # mm2 always loads [128, 2(stride 256), 128] like mm1.
                cpad = MTs[s] * 128
                aThi = [atp.tile([128, 2, cpad], dt.float8e4, tag=f"aThi_{s}_{v}",
                                 name=f"aThi_{s}_{v}") for v in range(3)]
                aTlo = [atp.tile([128, 2, cpad], dt.float8e4, tag=f"aTlo_{s}_{v}",
                                 name=f"aTlo_{s}_{v}") for v in range(3)]
                for v in range(3):
                    ups = psh.tile([128, 2, 512], dt.float32, tag=f"hp{v}",
                                   name=f"u{v}_{s}")
                    mm1_pair(s, ups, 3 + v)
                    # stt's stay on DVE so they stream with the parts; the
                    # hi/lo split runs on Pool for pairs 0,1 and on DVE for
                    # the critical last pair (no cross-engine hop before mm2)
                    nc.vector.scalar_tensor_tensor(
                        as2s[v][:], sils[v][:], S_A, ups[:, :, 0:c],
                        op0=mybir.AluOpType.mult, op1=mybir.AluOpType.mult)
                    eng = nc.vector if v == 2 else nc.gpsimd
                    eng.tensor_copy(aThi[v][:, :, 0:c], as2s[v][:])
                    eng.tensor_tensor(
                        aTlo[v][:, :, 0:c], as2s[v][:], aThi[v][:, :, 0:c],
                        op=mybir.AluOpType.subtract)
                # all 8 PSUM banks as 4 pair-tiles: group (m, dq) lives in
                # pair 2*m + dq//2, bank dq%2 — so the PSUM->SBUF copies can
                # be 1024-wide pair reads (half the copies and sem hops).
                # Tokens beyond c are garbage fp8; their PSUM rows are never
                # copied out.
                yhp = [psh.tile([128, 2, 512], dt.float32, tag=f"hp{p}",
                                name=f"yhp{p}_{s}") for p in range(4)]
                for p in range(W2P):
                    for m in range(MTs[s]):
                        for dq in range(4):
                            yh = yhp[2 * m + dq // 2][:, dq % 2, :]
                            for at in (aThi[p], aTlo[p]):
                                nc.tensor.matmul(
                                    yh, at[:, :, m * 128:(m + 1) * 128],
                                    w2t[s][p][:, 0:2, dq * 512:(dq + 1) * 512],
                                    start=(p == 0 and at is aThi[p]),
                                    stop=(p == W2P - 1 and at is aTlo[p]),
                                    perf_mode=mybir.MatmulPerfMode.DoubleRow)
                # 1024-wide pair copies on ACT/DVE (GPSIMD cannot access
                # PSUM), m0-first so each ye chunk DMA fires earliest.  On
                # the tail slot DVE (slower) takes the earlier-stopping dqp0
                # pairs so both engines stream without waiting.
                for pi in range(2 * MTs[s]):
                    m, dqp = pi // 2, pi % 2
                    rows = min(128, caps[s] - 128 * m)
                    cg = combg[0:rows, mi + m:mi + m + 1]
                    dst = yes[s][0:rows, m, dqp * 1024:(dqp + 1) * 1024]
                    src = yhp[2 * m + dqp][0:rows]
                    on_act = (pi % 2 == 1) if s == S - 1 else (pi % 2 == 0)
                    if on_act:
                        nc.scalar.activation(dst, src, AF.Copy, scale=cg)
                    else:
                        nc.vector.tensor_scalar(dst, src, cg, None,
                                                op0=mybir.AluOpType.mult)
                if s == S - 1:
                    # Tail slot: half-D transfers wait only their own pair
                    # copy, spread across queues so the issue latencies
                    # overlap (config picked by sim sweep; "-" merges the
                    # transfer with the previous one on that queue)
                    tq = [{"sp": nc.sync, "act": nc.scalar,
                           "pool": nc.gpsimd}[q] for q in _TAIL_QUEUES]
                    for m in range(MTs[s]):
                        rows = min(128, caps[s] - 128 * m)
                        if _TAIL_MERGE[m % len(_TAIL_MERGE)]:
                            tq[2 * m + 1].dma_start(ye_d[mi + m, 0:rows, :],
                                                    yes[s][0:rows, m, :])
                        else:
                            for dqp in range(2):
                                tq[2 * m + dqp].dma_start(
                                    ye_d[mi + m, 0:rows,
                                         dqp * 1024:(dqp + 1) * 1024],
                                    yes[s][0:rows, m,
                                           dqp * 1024:(dqp + 1) * 1024])
                else:
                    for m in range(MTs[s]):
                        rows = min(128, caps[s] - 128 * m)
                        nc.sync.dma_start(ye_d[mi + m, 0:rows, :],
                                          yes[s][0:rows, m, :])
                mi += MTs[s]

    nc.finalize()
    if split_waits:
        _split_excess_waits(nc)
    return nc


def kernel(hidden_states, topk_weights, topk_ids, w13_weight, w13_weight_scale,
           w2_weight, w2_weight_scale):
    from concourse.bass_utils import run_bass_kernel_spmd

    x = np.asarray(hidden_states, dtype=np.float32)
    tw = np.asarray(topk_weights, dtype=np.float32)
    ti = np.asarray(topk_ids)

    # host routing: combine weights + per-expert token lists
    comb = np.zeros((T, E), np.float32)
    for k in range(TOPK):
        np.add.at(comb, (np.arange(T), ti[:, k]), tw[:, k])
    routed = comb > 0.0
    idx = [np.nonzero(routed[:, e])[0] for e in range(E)]

    # per-core slots: each slot is (expert, token subrange) with <= 256
    # tokens; slot 0 gets the larger share so caps stay tight.
    core_slots = []
    for core in range(N_CORES):
        slots = []
        for le in range(E_LOC):
            e = core * E_LOC + le
            n = len(idx[e])
            for st in range(0, max(n, 1), 256):
                slots.append((e, st, min(256, n - st)))
        slots.sort(key=lambda t: -t[2])
        core_slots.append(slots)
    NS = max(len(s) for s in core_slots)
    for slots in core_slots:
        while len(slots) < NS:
            slots.append((0, 0, 0))
    caps = tuple(
        max(16, -(-max(core_slots[c][s][2] for c in range(N_CORES)) // 8) * 8)
        for s in range(NS))

    if caps not in _PROGRAM_CACHE:
        _PROGRAM_CACHE[caps] = _build_program(caps)
    nc = _PROGRAM_CACHE[caps]

    SLOTS = sum(caps)
    OFF = [sum(caps[:s]) for s in range(NS)]
    MTs = [-(-c // 128) for c in caps]
    TOTM = sum(MTs)
    MOFF = [sum(MTs[:s]) for s in range(NS)]
    DT, IT = D // 128, I // 128

    # host staging quantization + gather/transpose + lossless weight conv
    xq8 = x.astype(FP8)
    w13 = _dequant_mxfp4(np.asarray(w13_weight), np.asarray(w13_weight_scale))
    w2 = _dequant_mxfp4(np.asarray(w2_weight), np.asarray(w2_weight_scale))
    # w13 parts are f-column-major k-pairs: part j = cols [256j, 256j+256)
    # of w13[e].T, laid out [128, DT, 256] partition-major
    W13P, FH = 6, 256
    w13t8 = [np.ascontiguousarray(
        w13[e].T.astype(FP8).reshape(DT, 128, W13P, FH).transpose(2, 1, 0, 3))
        for e in range(E)]
    w2t8 = [w2[e].T.astype(FP8).reshape(IT, 128, D) for e in range(E)]

    in_maps = []
    for core in range(N_CORES):
        m = {}
        xgt = np.zeros((128, DT, SLOTS), FP8)
        cg = np.zeros((128, TOTM), np.float32)
        w13m = np.zeros((NS, W13P, 128, DT, FH), FP8)
        w2m = np.zeros((NS, IT, 128, D), FP8)
        for s, (e, st, n) in enumerate(core_slots[core]):
            if n == 0:
                continue
            ix = idx[e][st:st + n]
            xgt[:, :, OFF[s]:OFF[s] + n] = np.transpose(
                xq8[ix].T.reshape(DT, 128, n), (1, 0, 2))
            cw = comb[ix, e] / S_A      # undo the fp8 pre-scale
            for mm in range(MTs[s]):
                r = min(128, n - 128 * mm)
                if r > 0:
                    cg[0:r, MOFF[s] + mm] = cw[128 * mm:128 * mm + r]
            w13m[s] = w13t8[e]
            w2m[s] = w2t8[e]
        m["xgt"] = xgt
        m["combg"] = cg
        m["w13t"] = w13m
        m["w2t"] = w2m
        in_maps.append(m)

    res = run_bass_kernel_spmd(nc, in_maps, list(range(N_CORES)))

    out = np.zeros((T, D), np.float32)
    for core in range(N_CORES):
        ye = np.asarray(res.results[core]["ye"], dtype=np.float32)
        for s, (e, st, n) in enumerate(core_slots[core]):
            if n == 0:
                continue
            ix = idx[e][st:st + n]
            for mm in range(MTs[s]):
                r = min(128, n - 128 * mm)
                if r > 0:
                    out[ix[128 * mm:128 * mm + r]] += ye[MOFF[s] + mm, 0:r]
    return out
